# revision 18
# baseline (speedup 1.0000x reference)
"""RBF (Gaussian) kernel Gram matrix on 8 Trainium2 NeuronCores.

out[i, j] = exp(-gamma * ||x_i - y_j||^2),  x, y: [8192, 256] fp32.

Strategy (data-parallel over rows of x; y replicated):
  - Rows of x sharded across 8 cores (1024 rows each). Each core computes its
    [1024, 8192] stripe as
        out = exp(2g*(x.y) - g*||x||^2) * exp(-g*||y||^2)
    PE does the k=256 fp16 GEMM (2 k-tiles, 4 PSUM banks per 2048-col group),
    ACT applies exp with the per-partition -g*||x||^2 bias straight out of
    PSUM (fp16 out), DVE multiplies by the per-column exp(-g*||y||^2) factor
    (fp16 tensor_tensor, 2x mode), DMA streams the fp16 stripe to DRAM.
  - The per-column factor arrives pre-broadcast from the host ([128, 8192]
    fp16, 2MB); its later groups load with ~15us of slack. A handful of
    throwaway k=1 matmuls at kernel start keep the PE busy so the HAM clock
    gate reaches 2.4GHz before the real GEMM begins.
  - fp16 output halves HBM write traffic; host casts back to fp32.
    Max rel err ~1.5e-3, well inside the 2e-2 gate.
"""

import numpy as np

GAMMA = 0.005
FULL_N = 8192
D = 256
N_CORES = 8
M_SHARD = FULL_N // N_CORES  # 1024 rows of x per core
P = 128
M_TILES = M_SHARD // P  # 8
GROUP = 2048  # columns of output per PSUM tile (4 banks)
BANK = 512  # fp32 columns per PSUM bank (one matmul's max free dim)
N_GROUPS = FULL_N // GROUP  # 4

_cache = {}


def _split_sync_waits(nc, maxw=1):
    """walrus codegen rejects instructions carrying more than ~2 sync waits
    ("Too many sync wait commands"). Tile can attach many (e.g. the tail
    drain waits on every semaphore; a matmul can wait on several DMA lanes).
    Hoist the excess onto wait-only EventSemaphore instructions inserted
    just before the offender on the same engine (engines execute their
    instructions in block order, so all waits still precede the op)."""
    import concourse.mybir as mybir

    n_new = 0
    for fn in nc.m.functions:
        for bb in fn.blocks:
            insts = bb.instructions
            if not any(
                i.sync_info is not None and len(i.sync_info.on_wait) > maxw
                for i in insts
            ):
                continue
            new = []
            for inst in insts:
                si = inst.sync_info
                if si is not None and len(si.on_wait) > maxw:
                    waits = list(si.on_wait)
                    for i in range(0, len(waits) - maxw, maxw):
                        ev = mybir.InstEventSemaphore(
                            name=f"wsplit_{n_new}", ins=[], outs=[]
                        )
                        n_new += 1
                        ev.engine = inst.engine
                        ev.sync_info = mybir.SyncInfo(
                            on_wait=waits[i : i + maxw], on_update=[]
                        )
                        new.append(ev)
                    si.on_wait = waits[len(waits) - maxw :]
                new.append(inst)
            bb.instructions = new


def _build():
    import concourse.bass as bass
    import concourse.mybir as mybir
    import concourse.tile as tile

    f32 = mybir.dt.float32
    f16 = mybir.dt.float16
    nc = bass.Bass("TRN2", target_bir_lowering=False, debug=False)
    xt = nc.dram_tensor("xt", [D, M_SHARD], f16, kind="ExternalInput").ap()
    yt = nc.dram_tensor("yt", [D, FULL_N], f16, kind="ExternalInput").ap()
    x2 = nc.dram_tensor("x2", [P, M_TILES], f32, kind="ExternalInput").ap()
    eyr = nc.dram_tensor("eyr", [1, FULL_N], f16, kind="ExternalInput").ap()
    out = nc.dram_tensor("out", [M_SHARD, FULL_N], f16, kind="ExternalOutput").ap()

    with tile.TileContext(nc) as tc:
        with (
            tc.tile_pool(name="const", bufs=1) as cpool,
            tc.tile_pool(name="actp", bufs=3) as apool,
            tc.tile_pool(name="outp", bufs=4) as opool,
            tc.tile_pool(name="psum", bufs=2, space="PSUM") as ppool,
        ):
            ones = cpool.tile([1, P], f16, tag="ones")
            nc.any.memset(ones, 1.0)
            # Preload the ACT exp table set (~1.3us) off the critical path:
            # the first real ACTIVATE would otherwise trigger it lazily.
            tldw = cpool.tile([1, 1], f16, tag="tldw")
            nc.scalar.activation(
                tldw, ones[:, 0:1], mybir.ActivationFunctionType.Exp
            )
            # Input loads, most-urgent first: the first PSUM group needs xt
            # and the first 2048 columns of both y k-tiles (as 1024-col
            # halves so the first matmuls start as soon as each lands).
            xt0 = cpool.tile([P, M_SHARD], f16, tag="xt0")
            xt1 = cpool.tile([P, M_SHARD], f16, tag="xt1")
            x2sb = cpool.tile([P, M_TILES], f32, tag="x2")
            yt0 = cpool.tile([P, FULL_N], f16, tag="yt0")
            yt1 = cpool.tile([P, FULL_N], f16, tag="yt1")
            ey = cpool.tile([P, FULL_N], f16, tag="ey")
            HALF = GROUP // 2
            nc.sync.dma_start(out=xt0, in_=xt[0:P, :])
            for b in range(2):
                sl = slice(b * HALF, (b + 1) * HALF)
                nc.sync.dma_start(out=yt0[:, sl], in_=yt[0:P, sl])
                if b == 0:
                    nc.sync.dma_start(out=xt1, in_=xt[P : 2 * P, :])
                    nc.sync.dma_start(out=x2sb, in_=x2)
                nc.sync.dma_start(out=yt1[:, sl], in_=yt[P : 2 * P, sl])
            # exp(-g*||y||^2) arrives as a single 16KB row; DMAs with a
            # broadcast (partition-stride-0) DRAM source replicate it to all
            # 128 partitions. The re-reads hit a 16KB hot region, so the HBM
            # cost is far below a streaming 2MB load. Group 0 is needed by
            # the first DVE mult, so it goes before the yt bulk; groups 1-3
            # are only needed ~15us/group into the pipeline.
            nc.sync.dma_start(
                out=ey[:, 0:GROUP], in_=eyr[:, 0:GROUP].to_broadcast((P, GROUP))
            )
            for g in range(1, N_GROUPS):
                sl = slice(g * GROUP, (g + 1) * GROUP)
                nc.sync.dma_start(out=yt0[:, sl], in_=yt[0:P, sl])
                nc.sync.dma_start(out=yt1[:, sl], in_=yt[P : 2 * P, sl])
            for g in range(1, N_GROUPS):
                sl = slice(g * GROUP, (g + 1) * GROUP)
                nc.sync.dma_start(
                    out=ey[:, sl], in_=eyr[:, sl].to_broadcast((P, GROUP))
                )

            # HAM warmup: PE activity flips the clock gate from 1.2GHz to
            # 2.4GHz after a ~3.4us-busy window. Bare LDWEIGHTS (107ns each,
            # no PSUM writes, no data deps beyond the memset) keep the PE
            # densely busy from ~8us while the input slivers land, so the
            # real matmuls start at (or quickly reach) full clock.
            for w in range(34):
                nc.tensor.ldweights(ones)

            # Main loop, g-major.
            for g in range(N_GROUPS):
                for t in range(M_TILES):
                    msl = slice(t * P, (t + 1) * P)
                    gsl = slice(g * GROUP, (g + 1) * GROUP)
                    ps = ppool.tile([P, GROUP], f32, tag="ps")
                    for d, (xtd, ytd) in enumerate(((xt0, yt0), (xt1, yt1))):
                        for b in range(GROUP // BANK):
                            nsl = slice(
                                g * GROUP + b * BANK, g * GROUP + (b + 1) * BANK
                            )
                            bsl = slice(b * BANK, (b + 1) * BANK)
                            nc.tensor.matmul(
                                ps[:, bsl], xtd[:, msl], ytd[:, nsl],
                                start=(d == 0), stop=(d == 1),
                            )
                    at = apool.tile([P, GROUP], f16, tag="at")
                    # exp(2g*(x.y) - g*||x||^2): bias is per-partition, free
                    # on the ACT datapath.
                    nc.scalar.activation(
                        at, ps, mybir.ActivationFunctionType.Exp,
                        bias=x2sb[:, t : t + 1], scale=2.0 * GAMMA,
                    )
                    ot = opool.tile([P, GROUP], f16, tag="ot")
                    nc.vector.tensor_mul(ot, at, ey[:, gsl])
                    nc.sync.dma_start(out=out[msl, gsl], in_=ot)

    _split_sync_waits(nc, maxw=1)
    return nc


def kernel(x: np.ndarray, y: np.ndarray) -> np.ndarray:
    from concourse import bass_utils

    x = np.asarray(x, dtype=np.float32)
    y = np.asarray(y, dtype=np.float32)

    if "nc" not in _cache:
        _cache["nc"] = _build()
    nc = _cache["nc"]

    yt = np.ascontiguousarray(y.T.astype(np.float16))  # [256, 8192]
    xt_full = x.T.astype(np.float16)  # [256, 8192]
    x2 = np.sum(x.astype(np.float64) * x.astype(np.float64), axis=1)  # [8192]
    y2 = np.sum(y.astype(np.float64) * y.astype(np.float64), axis=1)
    eyr = np.exp(-GAMMA * y2).astype(np.float16).reshape(1, FULL_N)

    in_maps = []
    for c in range(N_CORES):
        cols = slice(c * M_SHARD, (c + 1) * M_SHARD)
        x2c = (-GAMMA * x2[cols]).astype(np.float32)
        in_maps.append(
            {
                "xt": np.ascontiguousarray(xt_full[:, cols]),
                "yt": yt,
                "x2": np.ascontiguousarray(x2c.reshape(M_TILES, P).T),
                "eyr": eyr,
            }
        )

    res = bass_utils.run_bass_kernel_spmd(
        nc, in_maps, core_ids=list(range(N_CORES))
    )
    _cache["last_result"] = res
    return np.concatenate(
        [res.results[c]["out"] for c in range(N_CORES)], axis=0
    ).astype(np.float32)


# revision 19
# speedup vs baseline: 1.0579x; 1.0579x over previous
"""RBF (Gaussian) kernel Gram matrix on 8 Trainium2 NeuronCores.

out[i, j] = exp(-gamma * ||x_i - y_j||^2),  x, y: [8192, 256] fp32.

Strategy (data-parallel over rows of x; y replicated):
  - Rows of x sharded across 8 cores (1024 rows each). Each core computes its
    [1024, 8192] stripe as
        out = exp(2g*(x.y) - g*||x||^2) * exp(-g*||y||^2)
    PE does the k=256 fp16 GEMM (2 k-tiles, 4 PSUM banks per 2048-col group),
    ACT applies exp with the per-partition -g*||x||^2 bias straight out of
    PSUM (fp16 out), DVE multiplies by the per-column exp(-g*||y||^2) factor
    (fp16 tensor_tensor, 2x mode), DMA streams the fp16 stripe to DRAM.
  - The per-column factor arrives pre-broadcast from the host ([128, 8192]
    fp16, 2MB); its later groups load with ~15us of slack. A handful of
    throwaway k=1 matmuls at kernel start keep the PE busy so the HAM clock
    gate reaches 2.4GHz before the real GEMM begins.
  - fp16 output halves HBM write traffic; host casts back to fp32.
    Max rel err ~1.5e-3, well inside the 2e-2 gate.
"""

import numpy as np

GAMMA = 0.005
FULL_N = 8192
D = 256
N_CORES = 8
M_SHARD = FULL_N // N_CORES  # 1024 rows of x per core
P = 128
M_TILES = M_SHARD // P  # 8
GROUP = 2048  # columns of output per PSUM tile (4 banks)
BANK = 512  # fp32 columns per PSUM bank (one matmul's max free dim)
N_GROUPS = FULL_N // GROUP  # 4

_cache = {}


def _split_sync_waits(nc, maxw=1):
    """walrus codegen rejects instructions carrying more than ~2 sync waits
    ("Too many sync wait commands"). Tile can attach many (e.g. the tail
    drain waits on every semaphore; a matmul can wait on several DMA lanes).
    Hoist the excess onto wait-only EventSemaphore instructions inserted
    just before the offender on the same engine (engines execute their
    instructions in block order, so all waits still precede the op)."""
    import concourse.mybir as mybir

    n_new = 0
    for fn in nc.m.functions:
        for bb in fn.blocks:
            insts = bb.instructions
            if not any(
                i.sync_info is not None and len(i.sync_info.on_wait) > maxw
                for i in insts
            ):
                continue
            new = []
            for inst in insts:
                si = inst.sync_info
                if si is not None and len(si.on_wait) > maxw:
                    waits = list(si.on_wait)
                    for i in range(0, len(waits) - maxw, maxw):
                        ev = mybir.InstEventSemaphore(
                            name=f"wsplit_{n_new}", ins=[], outs=[]
                        )
                        n_new += 1
                        ev.engine = inst.engine
                        ev.sync_info = mybir.SyncInfo(
                            on_wait=waits[i : i + maxw], on_update=[]
                        )
                        new.append(ev)
                    si.on_wait = waits[len(waits) - maxw :]
                new.append(inst)
            bb.instructions = new


def _build():
    import concourse.bass as bass
    import concourse.mybir as mybir
    import concourse.tile as tile

    f32 = mybir.dt.float32
    f16 = mybir.dt.float16
    nc = bass.Bass("TRN2", target_bir_lowering=False, debug=False)
    xt = nc.dram_tensor("xt", [D, M_SHARD], f16, kind="ExternalInput").ap()
    yt = nc.dram_tensor("yt", [D, FULL_N], f16, kind="ExternalInput").ap()
    x2 = nc.dram_tensor("x2", [P, M_TILES], f32, kind="ExternalInput").ap()
    eyr = nc.dram_tensor("eyr", [1, FULL_N], f16, kind="ExternalInput").ap()
    out = nc.dram_tensor("out", [M_SHARD, FULL_N], f16, kind="ExternalOutput").ap()

    with tile.TileContext(nc) as tc:
        with (
            tc.tile_pool(name="const", bufs=1) as cpool,
            tc.tile_pool(name="actp", bufs=3) as apool,
            tc.tile_pool(name="outp", bufs=4) as opool,
            tc.tile_pool(name="psum", bufs=2, space="PSUM") as ppool,
        ):
            ones = cpool.tile([1, P], f16, tag="ones")
            nc.any.memset(ones, 1.0)
            # Preload the ACT exp table set (~1.3us) off the critical path:
            # the first real ACTIVATE would otherwise trigger it lazily.
            tldw = cpool.tile([1, 1], f16, tag="tldw")
            nc.scalar.activation(
                tldw, ones[:, 0:1], mybir.ActivationFunctionType.Exp
            )
            # Input loads, most-urgent first: the first PSUM group needs xt
            # and the first 2048 columns of both y k-tiles (as 1024-col
            # halves so the first matmuls start as soon as each lands).
            xt0 = cpool.tile([P, M_SHARD], f16, tag="xt0")
            xt1 = cpool.tile([P, M_SHARD], f16, tag="xt1")
            x2sb = cpool.tile([P, M_TILES], f32, tag="x2")
            yt0 = cpool.tile([P, FULL_N], f16, tag="yt0")
            yt1 = cpool.tile([P, FULL_N], f16, tag="yt1")
            ey = cpool.tile([P, FULL_N], f16, tag="ey")
            eyr_sb = cpool.tile([1, FULL_N], f16, tag="eyr_sb")
            HALF = GROUP // 2
            # The 16KB ey row first: it feeds the warmup/broadcast matmuls.
            nc.sync.dma_start(out=eyr_sb, in_=eyr)
            nc.sync.dma_start(out=xt0, in_=xt[0:P, :])
            for b in range(2):
                sl = slice(b * HALF, (b + 1) * HALF)
                nc.sync.dma_start(out=yt0[:, sl], in_=yt[0:P, sl])
                if b == 0:
                    nc.sync.dma_start(out=xt1, in_=xt[P : 2 * P, :])
                    nc.sync.dma_start(out=x2sb, in_=x2)
                nc.sync.dma_start(out=yt1[:, sl], in_=yt[P : 2 * P, sl])
            for g in range(1, N_GROUPS):
                sl = slice(g * GROUP, (g + 1) * GROUP)
                nc.sync.dma_start(out=yt0[:, sl], in_=yt[0:P, sl])
                nc.sync.dma_start(out=yt1[:, sl], in_=yt[P : 2 * P, sl])
            # ey groups 1-3: DMAs with a broadcast (partition-stride-0) DRAM
            # source replicate the row to all 128 partitions. Issued last -
            # group g is only needed ~15us/group into the pipeline.
            for g in range(1, N_GROUPS):
                sl = slice(g * GROUP, (g + 1) * GROUP)
                nc.sync.dma_start(
                    out=ey[:, sl], in_=eyr[:, sl].to_broadcast((P, GROUP))
                )

            # HAM warmup: only MATMUL activity flips the PE clock gate from
            # 1.2GHz to 2.4GHz (takes a ~3.4us-busy window). Two junk k=1
            # matmuls (never read; WAW into a rotating psum tile is safe)
            # start as soon as the ey row lands, then the ey group-0
            # partition-broadcast runs as 4 more k=1 ones-matmuls whose PSUM
            # result the DVE casts to fp16 - useful work that keeps the PE
            # busy until the first yt slivers arrive.
            psw = ppool.tile([P, GROUP], f32, tag="ps")
            for w in range(2):
                nc.tensor.matmul(
                    psw[:, 0:BANK], ones, eyr_sb[:, 0:BANK],
                    start=True, stop=True,
                )
            psb = ppool.tile([P, GROUP], f32, tag="ps")
            for b in range(GROUP // BANK):
                bsl = slice(b * BANK, (b + 1) * BANK)
                nc.tensor.matmul(
                    psb[:, bsl], ones, eyr_sb[:, bsl], start=True, stop=True
                )
            nc.vector.tensor_copy(ey[:, 0:GROUP], psb)

            # Main loop, g-major.
            for g in range(N_GROUPS):
                for t in range(M_TILES):
                    msl = slice(t * P, (t + 1) * P)
                    gsl = slice(g * GROUP, (g + 1) * GROUP)
                    ps = ppool.tile([P, GROUP], f32, tag="ps")
                    for d, (xtd, ytd) in enumerate(((xt0, yt0), (xt1, yt1))):
                        for b in range(GROUP // BANK):
                            nsl = slice(
                                g * GROUP + b * BANK, g * GROUP + (b + 1) * BANK
                            )
                            bsl = slice(b * BANK, (b + 1) * BANK)
                            nc.tensor.matmul(
                                ps[:, bsl], xtd[:, msl], ytd[:, nsl],
                                start=(d == 0), stop=(d == 1),
                            )
                    at = apool.tile([P, GROUP], f16, tag="at")
                    # exp(2g*(x.y) - g*||x||^2): bias is per-partition, free
                    # on the ACT datapath.
                    nc.scalar.activation(
                        at, ps, mybir.ActivationFunctionType.Exp,
                        bias=x2sb[:, t : t + 1], scale=2.0 * GAMMA,
                    )
                    ot = opool.tile([P, GROUP], f16, tag="ot")
                    nc.vector.tensor_mul(ot, at, ey[:, gsl])
                    nc.sync.dma_start(out=out[msl, gsl], in_=ot)

    _split_sync_waits(nc, maxw=1)
    return nc


def kernel(x: np.ndarray, y: np.ndarray) -> np.ndarray:
    from concourse import bass_utils

    x = np.asarray(x, dtype=np.float32)
    y = np.asarray(y, dtype=np.float32)

    if "nc" not in _cache:
        _cache["nc"] = _build()
    nc = _cache["nc"]

    yt = np.ascontiguousarray(y.T.astype(np.float16))  # [256, 8192]
    xt_full = x.T.astype(np.float16)  # [256, 8192]
    x2 = np.sum(x.astype(np.float64) * x.astype(np.float64), axis=1)  # [8192]
    y2 = np.sum(y.astype(np.float64) * y.astype(np.float64), axis=1)
    eyr = np.exp(-GAMMA * y2).astype(np.float16).reshape(1, FULL_N)

    in_maps = []
    for c in range(N_CORES):
        cols = slice(c * M_SHARD, (c + 1) * M_SHARD)
        x2c = (-GAMMA * x2[cols]).astype(np.float32)
        in_maps.append(
            {
                "xt": np.ascontiguousarray(xt_full[:, cols]),
                "yt": yt,
                "x2": np.ascontiguousarray(x2c.reshape(M_TILES, P).T),
                "eyr": eyr,
            }
        )

    res = bass_utils.run_bass_kernel_spmd(
        nc, in_maps, core_ids=list(range(N_CORES))
    )
    _cache["last_result"] = res
    return np.concatenate(
        [res.results[c]["out"] for c in range(N_CORES)], axis=0
    ).astype(np.float32)
